# revision 71
# baseline (speedup 1.0000x reference)
"""MoE routing mixture kernel for Trainium2 (8 NeuronCores, SPMD).

Math: out[b] = sum_k selection_score[b, idx[b,k]] * all_weight[idx[b,k]]
Dense rewrite: out = C @ W_flat with
  C[b,e] = selection_score[b,e] * |{k : idx[b,k]==e}|   ([2048, 64])
  W_flat = all_weight.reshape(64, 16384)

Sharding (2 batch-groups x 4 column-groups): core c = 4*bg + cg handles
rows bg*1024:(bg+1)*1024 and cols cg*4096:(cg+1)*4096. W slice per core is
only [64, 4096] fp16; output dominates DMA traffic (~23.3us of ~25.5us at
the cost model's 360 B/ns serialized DMA bandwidth).

Everything on-chip is fp16 (PSUM accumulation stays f32): halves output
DMA bytes and runs matmuls at 1 cycle/row instead of f32's 4. idx values
ride in the fp16 input pack but are stored as f32 and accessed via AP
bitcast (is_equal needs an f32 scalar operand).

GPSIMD cannot touch PSUM on TRN2 (walrus verifier rejects it), so the
PSUM->SBUF fp16 cast drains live on ACT (~5 slices/rc) and DVE (~3);
GPSIMD instead computes the SBUF-only front half of C (is_equal onehots +
two tree-add levels) for chunks 1..7, keeping DVE under the ~2.9us/rc DMA
cadence. Chunk 0 is computed fully on DVE while GPSIMD/ACT are idle
(shortest pipeline fill).

Per-core pipeline (128-row chunk rc, 512-col slice s):
  SP  : 3 input DMAs (pack A, W, pack B), 22 output DMAs
  POOL: chunks 1-7: 8x is_equal onehot + 2 tree-add levels (SBUF only)
  DVE : chunk 0 full C; chunks 1-7 tail (add + count*score); C^T
        PSUM->SBUF copies; drains of slices (2,3) and 5
  PE  : warmup transposes (p-state ramp), 8 transposes C -> C^T (fp16
        PSUM), 64 matmuls [64,128]^T @ [64,512] -> f32 PSUM ring of 7
        one-bank slots
  ACT : drains of slices (0,1), 4, (6,7)
"""

import sys
from contextlib import ExitStack

import numpy as np

sys.path.insert(0, "/opt/trn_rl_repo")

BS, E, TOPK, PL, D = 2048, 64, 8, 32, 512
NF = PL * D  # 16384
N_CORES = 8
BG, CG = 2, 4  # batch groups x column groups
RPC = BS // BG  # 1024 rows per core
CPC = NF // CG  # 4096 cols per core
RCH = RPC // 128  # 8 row chunks
SLICES = CPC // D  # 8 matmul slices of 512 per row chunk
NSLOT = 7  # psum ring slots (one bank each)


# PSUM drain granules per rc: (s0, s1, engine). ACT takes 5 slices, DVE 3.
# rc0 uses single-slice ACT drains up front (earliest first output DMA);
# the last rc splits its tail pair (shortest tail). Granules whose slices
# would wrap the 7-slot ring are split into singles.
def drain_segs(rc):
    if rc == 0:
        base = [(0, 1, "A"), (1, 2, "A"), (2, 4, "V"), (4, 5, "A"), (5, 6, "V"),
                (6, 8, "A")]
    elif rc == RCH - 1:
        base = [(0, 2, "A"), (2, 4, "V"), (4, 5, "A"), (5, 6, "V"), (6, 7, "A"),
                (7, 8, "A")]
    else:
        base = [(0, 2, "A"), (2, 4, "V"), (4, 5, "A"), (5, 6, "V"), (6, 8, "A")]
    out = []
    for s0, s1, e in base:
        if (SLICES * rc + s0) % NSLOT + (s1 - s0) > NSLOT:
            out.extend((s, s + 1, e) for s in range(s0, s1))
        else:
            out.append((s0, s1, e))
    return out


# per-engine cumulative drain count after the seg covering slice q
_DRAIN_SEQ = {}


def _build_drain_seq():
    counts = {"A": 0, "V": 0}
    for rc in range(RCH):
        for s0, s1, e in drain_segs(rc):
            counts[e] += 1
            for s in range(s0, s1):
                _DRAIN_SEQ[SLICES * rc + s] = (e, counts[e])


_build_drain_seq()

_cache: dict = {}


def _drain_seq(q):
    """(engine, per-engine completion count) whose drain covers slice q."""
    return _DRAIN_SEQ[q]


def _build_program():
    import concourse.bass as bass
    import concourse.mybir as mybir

    f16 = mybir.dt.float16
    f32 = mybir.dt.float32
    eq = mybir.AluOpType.is_equal
    nc = bass.Bass()

    # pack, critical-first: sc0(64)|idx0(16)|sc1(64)|idx1(16)|iota(64)|
    # ident(128) || sc2-7(384)|idx2-7(96); DMA'd as [0:352) then [352:832)
    pack_d = nc.declare_dram_parameter("pack", [128, 832], f16, isOutput=False)
    w_d = nc.declare_dram_parameter("wk", [E, CPC], f16, isOutput=False)
    out_d = nc.declare_dram_parameter("out", [RPC, CPC], f16, isOutput=True)

    ctx = ExitStack()
    with ctx:
        sb = lambda tag, shape, dt=f16: ctx.enter_context(  # noqa: E731
            nc.sbuf_tensor(tag, shape, dt)
        )
        pack_t = sb("pack_t", [128, 832])
        w_t = sb("w_t", [E, CPC])
        eq_t = sb("eq_t", [128, TOPK * E])
        tmp2 = sb("tmp2", [128, 4 * E])
        tmp1 = [sb(f"tmp1_{i}", [128, 2 * E]) for i in range(4)]
        cnt_t = sb("cnt_t", [128, E])
        # DVE-only scratch for chunks 0-1 (GPSIMD path uses eq_t/tmp2/tmp1)
        veq_t = sb("veq_t", [128, TOPK * E])
        vt2 = sb("vt2", [128, 4 * E])
        vt1 = sb("vt1", [128, 2 * E])
        c_t = [sb(f"c{i}", [128, E]) for i in range(4)]
        ct_sb = sb("ct_sb", [E, RCH * 128])
        outimg = sb("outimg", [128, RCH * CPC])

        iota_ap = pack_t[:, 160:224]
        ident_ap = pack_t[:, 224:352]

        def sc_ap(r):
            if r == 0:
                return pack_t[:, 0:64]
            if r == 1:
                return pack_t[:, 80:144]
            return pack_t[:, 224 + r * 64 : 288 + r * 64]

        def ix_ap(r):
            lo = 64 if r == 0 else 144 if r == 1 else 704 + r * 16
            return pack_t[:, lo : lo + 16].bitcast(f32)

        mm_ps = ctx.enter_context(nc.psum_tensor("mm_ps", [128, NSLOT * D], f32))
        tp_ps = ctx.enter_context(nc.psum_tensor("tp_ps", [E, 4 * 128], f16))

        s_in = ctx.enter_context(nc.semaphore("s_in"))
        s_in2 = ctx.enter_context(nc.semaphore("s_in2"))
        s_w = ctx.enter_context(nc.semaphore("s_w"))
        s_c = ctx.enter_context(nc.semaphore("s_c"))
        s_pq = ctx.enter_context(nc.semaphore("s_pq"))  # gpsimd C front-half
        s_tp = ctx.enter_context(nc.semaphore("s_tp"))
        s_ct = ctx.enter_context(nc.semaphore("s_ct"))
        s_mm = ctx.enter_context(nc.semaphore("s_mm"))
        s_ad = ctx.enter_context(nc.semaphore("s_ad"))
        s_vd = ctx.enter_context(nc.semaphore("s_vd"))
        s_out = ctx.enter_context(nc.semaphore("s_out"))
        DSEM = {"A": s_ad, "V": s_vd}

        block = ctx.enter_context(nc.Block())

        # output DMA granules (rc0 fine-grained for earliest launch, rc7
        # split for shortest tail)
        def granules():
            for rc in range(RCH):
                if rc == 0:
                    segs = ((0, 1), (1, 2), (2, 4), (4, 6), (6, 8))
                elif rc == 1:
                    segs = ((0, 2), (2, 4), (4, 8))
                elif rc == RCH - 1:
                    segs = ((0, 4), (4, 6), (6, 7), (7, 8))
                else:
                    segs = ((0, 4), (4, 8))
                for seg in segs:
                    yield rc, seg

        N_GRAN = 5 + 3 + 2 * (RCH - 3) + 4

        @block.sync
        def _(sp):
            sp.dma_start(out=pack_t[:, 0:352], in_=pack_d[:, 0:352]).then_inc(s_in, 16)
            sp.dma_start(out=w_t[:], in_=w_d[:]).then_inc(s_w, 16)
            sp.dma_start(out=pack_t[:, 352:832], in_=pack_d[:, 352:832]).then_inc(
                s_in2, 16
            )
            for rc, (s0, s1) in granules():
                need = {}
                for s in range(s0, s1):
                    e, cntr = _drain_seq(SLICES * rc + s)
                    need[e] = max(need.get(e, 0), cntr)
                for e, n in sorted(need.items()):
                    sp.wait_ge(DSEM[e], n)
                rows = slice(rc * 128, (rc + 1) * 128)
                cols = slice(s0 * D, s1 * D)
                icols = slice(rc * CPC + s0 * D, rc * CPC + s1 * D)
                sp.dma_start(out=out_d[rows, cols], in_=outimg[:, icols]).then_inc(
                    s_out, 16
                )
            sp.wait_ge(s_out, 16 * N_GRAN)

        @block.gpsimd
        def _(gp):
            # front half of C for chunks 1..7: onehots + 2 tree-add levels
            gp.wait_ge(s_in, 16)
            for r in range(1, RCH):
                if r == 2:
                    gp.wait_ge(s_in2, 16)
                if r >= 5:
                    gp.wait_ge(s_c, r - 3)  # tmp1 ring of 4 vs DVE tail
                ix = ix_ap(r)
                for k in range(TOPK):
                    gp.tensor_scalar(
                        eq_t[:, k * E : (k + 1) * E],
                        iota_ap,
                        ix[:, k : k + 1],
                        None,
                        eq,
                    )
                gp.drain()
                gp.tensor_add(tmp2[:], eq_t[:, : 4 * E], eq_t[:, 4 * E :])
                gp.drain()
                gp.tensor_add(
                    tmp1[r % 4][:], tmp2[:, : 2 * E], tmp2[:, 2 * E :]
                ).then_inc(s_pq, 1)

        @block.vector
        def _(v):
            def c_full(r):
                # chunks 0-1: entire C on DVE (fill path, data in pack A)
                if r == 0:
                    v.wait_ge(s_in, 16)
                sc = sc_ap(r)
                ix = ix_ap(r)
                for k in range(TOPK):
                    v.tensor_scalar(
                        veq_t[:, k * E : (k + 1) * E],
                        iota_ap,
                        ix[:, k : k + 1],
                        None,
                        eq,
                    )
                v.drain()
                v.tensor_add(vt2[:], veq_t[:, : 4 * E], veq_t[:, 4 * E :])
                v.drain()
                v.tensor_add(vt1[:], vt2[:, : 2 * E], vt2[:, 2 * E :])
                v.drain()
                v.tensor_add(cnt_t[:], vt1[:, :E], vt1[:, E:])
                v.drain()
                v.tensor_mul(c_t[r % 4][:], cnt_t[:], sc).then_inc(s_c, 1)

            def c_tail(r):
                # chunks 1-7: last add + count*score from gpsimd's tmp1
                v.wait_ge(s_pq, r)
                if r >= 4:
                    v.wait_ge(s_tp, r - 3)  # c_t ring of 4
                t1 = tmp1[r % 4]
                v.tensor_add(cnt_t[:], t1[:, :E], t1[:, E:])
                v.drain()
                v.tensor_mul(c_t[r % 4][:], cnt_t[:], sc_ap(r)).then_inc(s_c, 1)

            def ct_drain(r):
                v.wait_ge(s_tp, r + 1)
                v.tensor_copy(
                    ct_sb[:, r * 128 : (r + 1) * 128],
                    tp_ps[:, (r % 4) * 128 : (r % 4 + 1) * 128],
                ).then_inc(s_ct, 1)

            def v_drain(rc, s0, s1):
                q1 = SLICES * rc + s1 - 1
                v.wait_ge(s_mm, q1 + 1)
                slot = (SLICES * rc + s0) % NSLOT
                v.tensor_copy(
                    outimg[:, rc * CPC + s0 * D : rc * CPC + s1 * D],
                    mm_ps[:, slot * D : (slot + s1 - s0) * D],
                ).then_inc(s_vd, 1)

            c_full(0)
            ct_drain(0)
            c_tail(1)
            ct_drain(1)
            for rc in range(RCH):
                # both drains first: they feed the psum ring (tight); the
                # c/ct pair feeds transposes 2 rcs ahead (slack). ct(j) must
                # not precede vd(rc,2,4) for j >= rc+2 (deadlock via T(j)).
                for s0, s1, e in drain_segs(rc):
                    if e == "V":
                        v_drain(rc, s0, s1)
                if rc + 2 < RCH:
                    c_tail(rc + 2)
                    ct_drain(rc + 2)

        @block.tensor
        def _(t):
            def tpose(r):
                t.wait_ge(s_c, r + 1)
                if r >= 4:
                    t.wait_ge(s_ct, r - 3)
                t.transpose(
                    tp_ps[:, (r % 4) * 128 : (r % 4 + 1) * 128],
                    c_t[r % 4][:],
                    ident_ap,
                ).then_inc(s_tp, 1)

            t.wait_ge(s_in, 16)
            # warm-up transposes: ramp the PE p-state while waiting for C0
            # (into tp slot 0, which T0 overwrites afterwards)
            for _ in range(5):
                t.transpose(tp_ps[:, 0:128], ident_ap[:, 0:64], ident_ap)
            tpose(0)
            for rc in range(RCH):
                t.wait_ge(s_ct, rc + 1)
                if rc == 0:
                    t.wait_ge(s_w, 16)
                for s in range(SLICES):
                    g = SLICES * rc + s
                    # T(rc+1) must precede mm(rc,3)/mm(rc,4), whose slot-free
                    # waits reference DVE drains queued behind ct(rc+1)
                    if s == 3 and rc + 1 < RCH:
                        tpose(rc + 1)
                    if g >= NSLOT:
                        e, cntr = _drain_seq(g - NSLOT)
                        t.wait_ge(DSEM[e], cntr)
                    slot = g % NSLOT
                    t.matmul(
                        mm_ps[:, slot * D : (slot + 1) * D],
                        ct_sb[:, rc * 128 : (rc + 1) * 128],
                        w_t[:, s * D : (s + 1) * D],
                        start=True,
                        stop=True,
                    ).then_inc(s_mm, 1)

        @block.scalar
        def _(a):
            for rc in range(RCH):
                for s0, s1, e in drain_segs(rc):
                    if e != "A":
                        continue
                    q1 = SLICES * rc + s1 - 1
                    a.wait_ge(s_mm, q1 + 1)
                    slot = (SLICES * rc + s0) % NSLOT
                    a.copy(
                        outimg[:, rc * CPC + s0 * D : rc * CPC + s1 * D],
                        mm_ps[:, slot * D : (slot + s1 - s0) * D],
                    ).then_inc(s_ad, 1)

    return nc


def _prep_inputs(selection_score, expert_indices, all_weight):
    scores = np.asarray(selection_score, dtype=np.float32)
    idx = np.asarray(expert_indices)
    w = np.asarray(all_weight, dtype=np.float32).reshape(E, NF).astype(np.float16)
    iota = np.tile(np.arange(E, dtype=np.float16), (128, 1))
    ident = np.eye(128, dtype=np.float16)

    in_maps = []
    for c in range(N_CORES):
        bg, cg = divmod(c, CG)
        sc = scores[bg * RPC : (bg + 1) * RPC]  # [1024, 64]
        ix = idx[bg * RPC : (bg + 1) * RPC]  # [1024, 8]
        # pack chunk-major: [128, rc*64+e] = sc[rc*128+p, e]
        sc_pk = (
            sc.reshape(RCH, 128, E).transpose(1, 0, 2).reshape(128, RCH * E)
        ).astype(np.float16)
        # idx stays f32 (is_equal scalar operand must be f32); shipped as
        # pairs of f16 columns inside the fp16 pack
        ix_pk = (
            ix.reshape(RCH, 128, TOPK).transpose(1, 0, 2).reshape(128, RCH * TOPK)
        ).astype(np.float32)
        ix_f16view = np.ascontiguousarray(ix_pk).view(np.float16)  # [128, 128]
        pack = np.ascontiguousarray(
            np.concatenate(
                [
                    sc_pk[:, :E],
                    ix_f16view[:, : 2 * TOPK],
                    sc_pk[:, E : 2 * E],
                    ix_f16view[:, 2 * TOPK : 4 * TOPK],
                    iota,
                    ident,
                    sc_pk[:, 2 * E :],
                    ix_f16view[:, 4 * TOPK :],
                ],
                axis=1,
            )
        )
        wk = np.ascontiguousarray(w[:, cg * CPC : (cg + 1) * CPC])
        in_maps.append({"pack": pack, "wk": wk})
    return in_maps


def _run(selection_score, expert_indices, all_weight, trace=False):
    from concourse.bass_utils import run_bass_kernel_spmd

    if "nc" not in _cache:
        _cache["nc"] = _build_program()
    nc = _cache["nc"]

    in_maps = _prep_inputs(selection_score, expert_indices, all_weight)
    r = run_bass_kernel_spmd(nc, in_maps, list(range(N_CORES)), trace=trace)
    full = np.empty((BS, NF), dtype=np.float32)
    for c in range(N_CORES):
        bg, cg = divmod(c, CG)
        full[bg * RPC : (bg + 1) * RPC, cg * CPC : (cg + 1) * CPC] = r.results[c][
            "out"
        ].astype(np.float32)
    return full.reshape(BS, PL, D), r


def kernel(selection_score, expert_indices, all_weight) -> np.ndarray:
    full, _ = _run(selection_score, expert_indices, all_weight, trace=False)
    return full
